# revision 20
# baseline (speedup 1.0000x reference)
"""Trainium2 Bass kernel for a 2-layer MoE decoder (moe_routing, 8 NeuronCores).

Strategy:
- Data-parallel trunk: each core owns 512 tokens (cores 0-3 = batch 0, cores 4-7 =
  batch 1; 4 contiguous 512-token chunks per batch). Per layer, K/V are
  AllGather'd within each batch group of 4 cores; attention/MoE/LN run locally.
- Top-2 expert routing depends only on c_states (a pure function of the inputs),
  so the router softmax/top-k is evaluated at input-prep time and only the two
  selected experts' weights are shipped/read (the moe_routing memory win); the
  blend weights are folded into w2/b2.
- Vocab head is tensor-parallel: final x is AllGather'd across all 8 cores and
  each core computes a 6283-column vocab shard of the logits for all 4096 tokens.
- bf16 matmuls with fp32 accumulation; residual stream + layernorms in fp32.
  Activations live transposed (xT: [D on partitions, tokens on free]) so the
  trunk needs no activation transposes. Attention uses a scoresT layout
  ([kv, q]) with a multiplicative causal mask after exp, and an appended
  ones-column on V so one matmul produces both attnT@V and the softmax
  denominator.
"""
import sys

sys.path.insert(0, "/opt/trn_rl_repo")

import numpy as np
import ml_dtypes

import concourse.bass as bass
import concourse.bacc as bacc
import concourse.mybir as mybir
import concourse.tile as tile
from concourse.bass_utils import run_bass_kernel_spmd
from concourse.masks import make_identity

BF16 = mybir.dt.bfloat16
F32 = mybir.dt.float32
I32 = mybir.dt.int32
AF = mybir.ActivationFunctionType
OP = mybir.AluOpType

NCORES = 8
P = 128
L, D, H, V, T, CDIM, E = 2, 1024, 16, 50257, 2048, 512, 8
FF = 4 * D
HD = 64
TC = 512          # tokens per core
NT = TC // P      # 4
ND = D // P       # 8
NKV = T // P      # 16
NF = FF // P      # 32
VS = 6283         # vocab shard (8*6283 = 50264)
NVC = 13          # 12*512 + 139
EPS = 1e-5

_prog_cache = {}
DEBUG_TAPS = False


def _build_program():
    if "nc" in _prog_cache:
        return _prog_cache["nc"]
    nc = bacc.Bacc(None, target_bir_lowering=False, num_devices=NCORES)

    toks_d = nc.dram_tensor("toks", [NT, P, 1], I32, kind="ExternalInput")
    embed_d = nc.dram_tensor("embed", [V, D], BF16, kind="ExternalInput")
    posT_d = nc.dram_tensor("posT", [D, TC], F32, kind="ExternalInput")
    mask_d = nc.dram_tensor("mask", [NKV, P, TC], BF16, kind="ExternalInput")
    headT_d = nc.dram_tensor("headT", [D, VS], BF16, kind="ExternalInput")
    lnfg_d = nc.dram_tensor("lnfg", [P, ND], F32, kind="ExternalInput")
    lnfb_d = nc.dram_tensor("lnfb", [P, ND], F32, kind="ExternalInput")
    lay = []
    for l in range(L):
        lay.append(dict(
            qkwT=nc.dram_tensor(f"qkwT{l}", [24, P, ND, P], BF16, kind="ExternalInput"),
            qkb=nc.dram_tensor(f"qkb{l}", [P, 24], F32, kind="ExternalInput"),
            outwT=nc.dram_tensor(f"outwT{l}", [ND, P, ND, P], BF16, kind="ExternalInput"),
            outb=nc.dram_tensor(f"outb{l}", [P, ND], F32, kind="ExternalInput"),
            lnag=nc.dram_tensor(f"lnag{l}", [P, ND], F32, kind="ExternalInput"),
            lnab=nc.dram_tensor(f"lnab{l}", [P, ND], F32, kind="ExternalInput"),
            w1T=nc.dram_tensor(f"w1T{l}", [2 * NF, P, ND, P], BF16, kind="ExternalInput"),
            b1=nc.dram_tensor(f"b1_{l}", [P, 2 * NF], F32, kind="ExternalInput"),
            w2T=nc.dram_tensor(f"w2T{l}", [ND, 2, 4, P, NF // 4, P], BF16, kind="ExternalInput"),
            b2=nc.dram_tensor(f"b2_{l}", [P, ND], F32, kind="ExternalInput"),
            lnmg=nc.dram_tensor(f"lnmg{l}", [P, ND], F32, kind="ExternalInput"),
            lnmb=nc.dram_tensor(f"lnmb{l}", [P, ND], F32, kind="ExternalInput"),
        ))

    logits_d = nc.dram_tensor("logits", [NCORES * TC, VS], BF16, kind="ExternalOutput")
    if DEBUG_TAPS:
        dbg0_d = nc.dram_tensor("dbg0", [D, TC], F32, kind="ExternalOutput")
        dbgf_d = nc.dram_tensor("dbgf", [D, TC], F32, kind="ExternalOutput")

    agk_in = [nc.dram_tensor(f"agk_in{l}", [D, TC], BF16) for l in range(L)]
    agk_out = [nc.dram_tensor(f"agk_out{l}", [4 * D, TC], BF16) for l in range(L)]
    agv_in = [nc.dram_tensor(f"agv_in{l}", [TC, D], BF16) for l in range(L)]
    agv_out = [nc.dram_tensor(f"agv_out{l}", [4 * TC, D], BF16) for l in range(L)]
    agx_in = nc.dram_tensor("agx_in", [D, TC], BF16)
    agx_out = nc.dram_tensor("agx_out", [NCORES * D, TC], BF16, addr_space="Shared")
    GROUPS4 = [[0, 1, 2, 3], [4, 5, 6, 7]]
    GROUPS8 = [list(range(NCORES))]

    with tile.TileContext(nc) as tc:
        with (
            tc.tile_pool(name="cst", bufs=1) as cst,
            tc.tile_pool(name="res", bufs=1) as res,
            tc.tile_pool(name="att", bufs=1) as att,
            tc.tile_pool(name="wrk", bufs=3) as wrk,
            tc.tile_pool(name="str", bufs=2) as strm,
            tc.tile_pool(name="psA", bufs=4, space="PSUM") as psA,
            tc.tile_pool(name="psB", bufs=2, space="PSUM") as psB,
            tc.tile_pool(name="psC", bufs=2, space="PSUM") as psC,
        ):
            # ---- constants ----
            ident = cst.tile([P, P], F32, name="ident", tag="ident")
            make_identity(nc, ident[:])
            ident_bf = cst.tile([P, P], BF16, name="ident_bf", tag="ident_bf")
            make_identity(nc, ident_bf[:])
            ones128 = cst.tile([P, 1], F32, name="ones128", tag="ones128")
            nc.vector.memset(ones128[:], 1.0)
            ones1r = cst.tile([1, P], F32, name="ones1r", tag="ones1r")
            nc.vector.memset(ones1r[:], 1.0)
            zcol = cst.tile([P, 1], F32, name="zcol", tag="zcol")
            nc.vector.memset(zcol[:], 0.0)
            epscol = cst.tile([1, 1], F32, name="epscol", tag="epscol")
            nc.vector.memset(epscol[:], EPS)

            def load_params(dram, nm, dt=F32):
                t = cst.tile(list(dram.shape), dt, name=nm, tag=nm)
                nc.sync.dma_start(t[:], dram[:, :])
                return t

            qkb_t = [load_params(lay[l]["qkb"], f"qkbt{l}") for l in range(L)]
            outb_t = [load_params(lay[l]["outb"], f"outbt{l}") for l in range(L)]
            lnag_t = [load_params(lay[l]["lnag"], f"lnagt{l}") for l in range(L)]
            lnab_t = [load_params(lay[l]["lnab"], f"lnabt{l}") for l in range(L)]
            b1_t = [load_params(lay[l]["b1"], f"b1t{l}") for l in range(L)]
            b2_t = [load_params(lay[l]["b2"], f"b2t{l}") for l in range(L)]
            lnmg_t = [load_params(lay[l]["lnmg"], f"lnmgt{l}") for l in range(L)]
            lnmb_t = [load_params(lay[l]["lnmb"], f"lnmbt{l}") for l in range(L)]
            lnfg_t = load_params(lnfg_d, "lnfgt")
            lnfb_t = load_params(lnfb_d, "lnfbt")

            masks = []
            for m in range(NKV):
                mt = cst.tile([P, TC], BF16, name=f"mask{m}", tag=f"mask{m}")
                nc.sync.dma_start(mt[:], mask_d[m, :, :])
                masks.append(mt)

            xres = [res.tile([P, TC], F32, name=f"xr{d}", tag=f"xr{d}") for d in range(ND)]
            x_bf = [res.tile([P, TC], BF16, name=f"xb{d}", tag=f"xb{d}") for d in range(ND)]

            def cast_resid_to_bf16():
                for d in range(ND):
                    nc.vector.tensor_copy(x_bf[d][:], xres[d][:])

            # ---------------- embedding + positions ----------------
            with tc.tile_pool(name="embp", bufs=1) as embp:
                for j in range(NT):
                    idx = embp.tile([P, 1], I32, name=f"idx{j}", tag="idx")
                    nc.sync.dma_start(idx[:], toks_d[j, :, :])
                    g = embp.tile([P, D], BF16, name=f"emb{j}", tag="emb")
                    nc.gpsimd.indirect_dma_start(
                        out=g[:], out_offset=None, in_=embed_d[:, :],
                        in_offset=bass.IndirectOffsetOnAxis(ap=idx[:, :1], axis=0),
                    )
                    gf = embp.tile([P, D], F32, name=f"embf{j}", tag="embf")
                    nc.vector.tensor_copy(gf[:], g[:])
                    for d in range(ND):
                        pt = embp.tile([P, P], F32, name=f"pos{j}_{d}", tag="pos")
                        nc.sync.dma_start(
                            pt[:], posT_d[d * P : (d + 1) * P, j * P : (j + 1) * P]
                        )
                        tp = psA.tile([P, P], F32, name=f"tp{j}_{d}", tag="a")
                        nc.tensor.transpose(tp[:], gf[:, d * P : (d + 1) * P], ident[:])
                        nc.vector.tensor_tensor(
                            xres[d][:, j * P : (j + 1) * P], tp[:], pt[:], OP.add
                        )
            cast_resid_to_bf16()

            # ---------------- layernorm helper ----------------
            ln_ctr = [0]

            def layernorm(g_tile, b_tile):
                i = ln_ctr[0]
                ln_ctr[0] += 1
                sum_ps = psC.tile([1, TC], F32, name=f"lnsum{i}", tag="c")
                for d in range(ND):
                    nc.tensor.matmul(
                        sum_ps[:], ones128[:], xres[d][:],
                        start=(d == 0), stop=(d == ND - 1),
                    )
                sq_ps = psC.tile([1, TC], F32, name=f"lnsq{i}", tag="c")
                for d in range(ND):
                    sq = wrk.tile([P, TC], F32, name=f"lnsqt{i}_{d}", tag="lnt", bufs=2)
                    nc.vector.tensor_tensor(sq[:], xres[d][:], xres[d][:], OP.mult)
                    nc.tensor.matmul(
                        sq_ps[:], ones128[:], sq[:], start=(d == 0), stop=(d == ND - 1)
                    )
                mu = wrk.tile([1, TC], F32, name=f"lnmu{i}", tag="lnmu", bufs=1)
                nc.vector.tensor_scalar_mul(mu[:], sum_ps[:], 1.0 / D)
                msq = wrk.tile([1, TC], F32, name=f"lnmsq{i}", tag="lnmsq", bufs=1)
                nc.vector.tensor_scalar_mul(msq[:], sq_ps[:], 1.0 / D)
                var = wrk.tile([1, TC], F32, name=f"lnvar{i}", tag="lnvar", bufs=1)
                nc.vector.tensor_tensor(var[:], mu[:], mu[:], OP.mult)
                nc.vector.tensor_tensor(var[:], msq[:], var[:], OP.subtract)
                sd = wrk.tile([1, TC], F32, name=f"lnsd{i}", tag="lnsd", bufs=1)
                nc.scalar.activation(sd[:], var[:], AF.Sqrt, bias=epscol[:], scale=1.0)
                rstd = wrk.tile([1, TC], F32, name=f"lnrstd{i}", tag="lnrstd", bufs=1)
                nc.vector.reciprocal(rstd[:], sd[:])
                t2 = wrk.tile([1, TC], F32, name=f"lnt2{i}", tag="lnt2", bufs=1)
                nc.vector.tensor_tensor(t2[:], mu[:], rstd[:], OP.mult)
                nc.vector.tensor_scalar_mul(t2[:], t2[:], -1.0)
                t1b = psC.tile([P, TC], F32, name=f"lnt1b{i}", tag="c")
                nc.tensor.matmul(t1b[:], ones1r[:, :], rstd[:], start=True, stop=True)
                t2b = psC.tile([P, TC], F32, name=f"lnt2b{i}", tag="c")
                nc.tensor.matmul(t2b[:], ones1r[:, :], t2[:], start=True, stop=True)
                for d in range(ND):
                    tn = wrk.tile([P, TC], F32, name=f"lnn{i}_{d}", tag="lnt", bufs=2)
                    nc.vector.tensor_tensor(tn[:], xres[d][:], t1b[:], OP.mult)
                    nc.vector.tensor_tensor(tn[:], tn[:], t2b[:], OP.add)
                    nc.vector.tensor_scalar(
                        xres[d][:], tn[:], g_tile[:, d : d + 1], b_tile[:, d : d + 1],
                        OP.mult, OP.add,
                    )
                cast_resid_to_bf16()

            # ---------------- transformer layers ----------------
            for l in range(L):
                # ---- q/k/v projection (transposed outputs) ----
                # o 0..7: qT tiles; o 8..15: kT tiles (-> AG); o 16..23: vT tiles,
                # PE-transposed into v [tok, feat] tiles (-> AG). Identity used
                # for the transposes; bias is per-partition in all three cases.
                vsb = [
                    att.tile([P, D], BF16, name=f"vsb{l}_{j}", tag=f"vsb{j}")
                    for j in range(NT)
                ]
                qT = [None] * 8
                for o in [*range(8, 16), *range(16, 24), *range(0, 8)]:
                    blk = strm.tile([P, ND, P], BF16, name=f"qkw{l}_{o}", tag="qkwblk", bufs=4)
                    nc.sync.dma_start(blk[:], lay[l]["qkwT"][o])
                    ps = psA.tile([P, TC], F32, name=f"qkps{l}_{o}", tag="a")
                    for d in range(ND):
                        nc.tensor.matmul(
                            ps[:], blk[:, d, :], x_bf[d][:],
                            start=(d == 0), stop=(d == ND - 1),
                        )
                    if o < 8:
                        qt = att.tile([P, TC], BF16, name=f"qt{l}_{o}", tag=f"qt{o}")
                        nc.vector.tensor_scalar(
                            qt[:], ps[:], qkb_t[l][:, o : o + 1], None, OP.add
                        )
                        qT[o] = qt
                    elif o < 16:
                        kt = wrk.tile([P, TC], BF16, name=f"ktc{l}_{o}", tag="ktc", bufs=2)
                        nc.vector.tensor_scalar(
                            kt[:], ps[:], qkb_t[l][:, o : o + 1], None, OP.add
                        )
                        nc.sync.dma_start(agk_in[l][(o - 8) * P : (o - 7) * P, :], kt[:])
                    else:
                        vt = wrk.tile([P, TC], BF16, name=f"vtc{l}_{o}", tag="ktc", bufs=2)
                        nc.vector.tensor_scalar(
                            vt[:], ps[:], qkb_t[l][:, o : o + 1], None, OP.add
                        )
                        for j in range(NT):
                            vtr = psB.tile([P, P], BF16, name=f"vtr{l}_{o}_{j}", tag="b")
                            nc.tensor.transpose(
                                vtr[:], vt[:, j * P : (j + 1) * P], ident_bf[:]
                            )
                            nc.vector.tensor_copy(
                                vsb[j][:, (o - 16) * P : (o - 15) * P], vtr[:]
                            )
                    if o == 15:
                        # K contributions complete: fire the K all-gather so it
                        # overlaps the V and Q projections.
                        nc.gpsimd.collective_compute(
                            "AllGather", OP.bypass, replica_groups=GROUPS4,
                            ins=[agk_in[l][:, :]], outs=[agk_out[l][:, :]],
                        )
                    if o == 23:
                        for j in range(NT):
                            nc.sync.dma_start(
                                agv_in[l][j * P : (j + 1) * P, :], vsb[j][:]
                            )
                        nc.gpsimd.collective_compute(
                            "AllGather", OP.bypass, replica_groups=GROUPS4,
                            ins=[agv_in[l][:, :]], outs=[agv_out[l][:, :]],
                        )

                # ---- v blocks as augmented tiles [tok 128, H, HD+1] ----
                vaug = []
                for m in range(NKV):
                    vt = att.tile([P, H, HD + 1], BF16, name=f"vaug{l}_{m}", tag=f"vaug{m}")
                    nc.vector.memset(vt[:], 1.0)
                    nc.sync.dma_start(
                        vt[:, :, 0:HD],
                        agv_out[l][m * P : (m + 1) * P, :].rearrange(
                            "p (h c) -> p h c", h=H
                        ),
                    )
                    vaug.append(vt)

                # ---- attention ----
                ao_t = [
                    att.tile([P, TC], BF16, name=f"ao{l}_{o}", tag=f"ao{o}")
                    for o in range(ND)
                ]
                for ft in range(ND):
                    kts = []
                    for c in range(4):
                        kt = strm.tile([P, TC], BF16, name=f"kt{l}_{ft}_{c}", tag=f"kt{c}", bufs=1)
                        nc.sync.dma_start(
                            kt[:], agk_out[l][c * D + ft * P : c * D + (ft + 1) * P, :]
                        )
                        kts.append(kt)
                    for hh in range(2):
                        h = 2 * ft + hh
                        rs = slice(HD * hh, HD * (hh + 1))
                        aog = psB.tile([HD + 1, TC], F32, name=f"aog{l}_{h}", tag="b")
                        for m in range(NKV):
                            c, b = m // 4, m % 4
                            sc = psA.tile([P, TC], F32, name=f"sc{l}_{h}_{m}", tag="a")
                            nc.tensor.matmul(
                                sc[:], kts[c][rs, b * P : (b + 1) * P], qT[ft][rs, :],
                                start=True, stop=True,
                            )
                            em = wrk.tile([P, TC], BF16, name=f"em{l}_{h}_{m}", tag="em", bufs=6)
                            nc.scalar.activation(
                                em[:], sc[:], AF.Exp, bias=zcol[:], scale=1.0
                            )
                            nc.vector.tensor_tensor(em[:], em[:], masks[m][:], OP.mult)
                            nc.tensor.matmul(
                                aog[:], vaug[m][:, h, :], em[:],
                                start=(m == 0), stop=(m == NKV - 1),
                            )
                        rcp = wrk.tile([1, TC], F32, name=f"rcp{l}_{h}", tag="rcp", bufs=1)
                        nc.vector.reciprocal(rcp[:], aog[HD : HD + 1, :])
                        dbc = psC.tile([HD, TC], F32, name=f"dbc{l}_{h}", tag="c")
                        nc.tensor.matmul(
                            dbc[:], ones1r[:, 0:HD], rcp[:], start=True, stop=True
                        )
                        dbs = wrk.tile([HD, TC], F32, name=f"dbs{l}_{h}", tag="dbs", bufs=2)
                        nc.scalar.activation(dbs[:], dbc[:], AF.Copy)
                        nc.vector.tensor_tensor(
                            ao_t[ft][rs, :], aog[0:HD, :], dbs[:], OP.mult
                        )

                # ---- output projection + residual ----
                for o in range(ND):
                    blk = strm.tile([P, ND, P], BF16, name=f"outw{l}_{o}", tag="outwblk", bufs=3)
                    nc.sync.dma_start(blk[:], lay[l]["outwT"][o])
                    ps = psA.tile([P, TC], F32, name=f"ops{l}_{o}", tag="a")
                    for d in range(ND):
                        nc.tensor.matmul(
                            ps[:], blk[:, d, :], ao_t[d][:],
                            start=(d == 0), stop=(d == ND - 1),
                        )
                    t = wrk.tile([P, TC], F32, name=f"ores{l}_{o}", tag="ores", bufs=2)
                    nc.vector.tensor_scalar(
                        t[:], ps[:], outb_t[l][:, o : o + 1], None, OP.add
                    )
                    nc.vector.tensor_tensor(xres[o][:], xres[o][:], t[:], OP.add)
                layernorm(lnag_t[l], lnab_t[l])

                if l == 0 and DEBUG_TAPS:
                    for d in range(ND):
                        nc.sync.dma_start(dbg0_d[d * P : (d + 1) * P, :], xres[d][:])

                # ---- MoE: two pre-selected experts (blend folded into w2/b2) ----
                for e in range(2):
                    hsb = []
                    for o2 in range(NF):
                        blk = strm.tile([P, ND, P], BF16, name=f"w1b{l}_{e}_{o2}", tag="w1blk", bufs=4)
                        nc.sync.dma_start(blk[:], lay[l]["w1T"][e * NF + o2])
                        ps = psA.tile([P, TC], F32, name=f"hps{l}_{e}_{o2}", tag="a")
                        for d in range(ND):
                            nc.tensor.matmul(
                                ps[:], blk[:, d, :], x_bf[d][:],
                                start=(d == 0), stop=(d == ND - 1),
                            )
                        ht = att.tile([P, TC], BF16, name=f"hsb{l}_{e}_{o2}", tag=f"hsb{o2}")
                        nc.scalar.activation(
                            ht[:], ps[:], AF.Gelu,
                            bias=b1_t[l][:, e * NF + o2 : e * NF + o2 + 1], scale=1.0,
                        )
                        hsb.append(ht)
                    for o in range(ND):
                        ps = psB.tile([P, TC], F32, name=f"yps{l}_{e}_{o}", tag="b")
                        for half in range(4):
                            w2b = strm.tile(
                                [P, NF // 4, P], BF16, name=f"w2b{l}_{e}_{o}_{half}", tag="w2blk"
                            )
                            nc.sync.dma_start(w2b[:], lay[l]["w2T"][o, e, half])
                            for oo in range(NF // 4):
                                o2 = half * (NF // 4) + oo
                                nc.tensor.matmul(
                                    ps[:], w2b[:, oo, :], hsb[o2][:],
                                    start=(o2 == 0), stop=(o2 == NF - 1),
                                )
                        if e == 0:
                            nc.vector.tensor_tensor(xres[o][:], xres[o][:], ps[:], OP.add)
                        else:
                            t = wrk.tile([P, TC], F32, name=f"ycmb{l}_{o}", tag="ycmb", bufs=2)
                            nc.vector.tensor_scalar(
                                t[:], ps[:], b2_t[l][:, o : o + 1], None, OP.add
                            )
                            nc.vector.tensor_tensor(xres[o][:], xres[o][:], t[:], OP.add)
                layernorm(lnmg_t[l], lnmb_t[l])

            # ---------------- final LN + x all-gather ----------------
            layernorm(lnfg_t, lnfb_t)
            for d in range(ND):
                if DEBUG_TAPS:
                    nc.sync.dma_start(dbgf_d[d * P : (d + 1) * P, :], xres[d][:])
                nc.sync.dma_start(agx_in[d * P : (d + 1) * P, :], x_bf[d][:])
            nc.gpsimd.collective_compute(
                "AllGather", OP.bypass, replica_groups=GROUPS8,
                ins=[agx_in[:, :]], outs=[agx_out[:, :]],
            )

        # ---------------- head phase (trunk pools released) ----------------
        with (
            tc.tile_pool(name="hw", bufs=1) as hwp,
            tc.tile_pool(name="hx", bufs=3) as hxp,
            tc.tile_pool(name="hwk", bufs=6) as hwrk,
            tc.tile_pool(name="hps", bufs=6, space="PSUM") as hps,
        ):
            hw_t = []
            for vc in range(NVC):
                w = min(TC, VS - vc * TC)
                row = []
                for d in range(ND):
                    t = hwp.tile([P, TC], BF16, name=f"hw{vc}_{d}", tag=f"hw{vc}_{d}")
                    nc.sync.dma_start(
                        t[:, 0:w], headT_d[d * P : (d + 1) * P, vc * TC : vc * TC + w]
                    )
                    row.append(t)
                hw_t.append(row)
            for r in range(NCORES):
                xa = []
                for d in range(ND):
                    t = hxp.tile([P, TC], BF16, name=f"xa{r}_{d}", tag=f"xa{d}")
                    nc.sync.dma_start(
                        t[:], agx_out[r * D + d * P : r * D + (d + 1) * P, :]
                    )
                    xa.append(t)
                for j in range(NT):
                    for vc in range(NVC):
                        w = min(TC, VS - vc * TC)
                        ps = hps.tile([P, TC], F32, name=f"lps{r}_{j}_{vc}", tag="lps")
                        for d in range(ND):
                            nc.tensor.matmul(
                                ps[:, 0:w], xa[d][:, j * P : (j + 1) * P],
                                hw_t[vc][d][:, 0:w],
                                start=(d == 0), stop=(d == ND - 1),
                            )
                        lt = hwrk.tile([P, TC], BF16, name=f"lt{r}_{j}_{vc}", tag="lt")
                        nc.scalar.activation(lt[:, 0:w], ps[:, 0:w], AF.Copy)
                        nc.sync.dma_start(
                            logits_d[
                                r * TC + j * P : r * TC + (j + 1) * P,
                                vc * TC : vc * TC + w,
                            ],
                            lt[:, 0:w],
                        )

    nc.compile()
    _prog_cache["nc"] = nc
    return nc


def _tile4(mat, no, nd):
    """[nd*P, no*P] -> [no, P, nd, P] with out[o, p, d, c] = mat[d*P+p, o*P+c]."""
    return np.ascontiguousarray(
        mat.reshape(nd, P, no, P).transpose(2, 1, 0, 3)
    )


def _prep_inputs(inputs):
    bf = ml_dtypes.bfloat16
    f32 = np.float32
    tokens = np.asarray(inputs["tokens"]).astype(np.int32)
    c_states = np.asarray(inputs["c_states"], dtype=f32)
    embed_w = np.asarray(inputs["embed_w"], dtype=f32)
    pos_w = np.asarray(inputs["pos_w"], dtype=f32)
    in_w = np.asarray(inputs["in_w"], dtype=f32)
    in_b = np.asarray(inputs["in_b"], dtype=f32)
    out_w = np.asarray(inputs["out_w"], dtype=f32)
    out_b = np.asarray(inputs["out_b"], dtype=f32)
    ln_a_g = np.asarray(inputs["ln_a_g"], dtype=f32)
    ln_a_b = np.asarray(inputs["ln_a_b"], dtype=f32)
    router_w = np.asarray(inputs["router_w"], dtype=f32)
    router_b = np.asarray(inputs["router_b"], dtype=f32)
    e_w1 = np.asarray(inputs["e_w1"], dtype=f32)
    e_b1 = np.asarray(inputs["e_b1"], dtype=f32)
    e_w2 = np.asarray(inputs["e_w2"], dtype=f32)
    e_b2 = np.asarray(inputs["e_b2"], dtype=f32)
    ln_m_g = np.asarray(inputs["ln_m_g"], dtype=f32)
    ln_m_b = np.asarray(inputs["ln_m_b"], dtype=f32)
    ln_f_g = np.asarray(inputs["ln_f_g"], dtype=f32)
    ln_f_b = np.asarray(inputs["ln_f_b"], dtype=f32)
    head_w = np.asarray(inputs["head_w"], dtype=f32)

    def colmaj(v):
        n = v.shape[0] // P
        return np.ascontiguousarray(v.reshape(n, P).T).astype(f32)

    shared = {"embed": embed_w.astype(bf)}
    for l in range(L):
        qk_w = in_w[l].copy()  # [3072, 1024]
        qk_b = in_b[l].copy()
        qk_w[:D] *= 1.0 / np.sqrt(HD)
        qk_b[:D] *= 1.0 / np.sqrt(HD)
        shared[f"qkwT{l}"] = _tile4(qk_w.T, 24, ND).astype(bf)
        shared[f"qkb{l}"] = colmaj(qk_b)
        shared[f"outwT{l}"] = _tile4(out_w[l].T, ND, ND).astype(bf)
        shared[f"outb{l}"] = colmaj(out_b[l])
        shared[f"lnag{l}"] = colmaj(ln_a_g[l])
        shared[f"lnab{l}"] = colmaj(ln_a_b[l])
        shared[f"lnmg{l}"] = colmaj(ln_m_g[l])
        shared[f"lnmb{l}"] = colmaj(ln_m_b[l])

        # routing: pure function of the inputs; ship only top-2 experts
        c_mean = c_states.mean(0)
        rl = router_w[l] @ c_mean + router_b[l]
        probs = np.exp(rl - rl.max())
        probs /= probs.sum()
        idx = np.argsort(-probs)[:2]
        w2top = probs[idx] / probs[idx].sum()

        w1stk = np.empty((2 * NF, P, ND, P), dtype=bf)
        b1cat = np.empty((P, 2 * NF), dtype=f32)
        for e in range(2):
            w1stk[e * NF : (e + 1) * NF] = _tile4(e_w1[l][idx[e]].T, NF, ND).astype(bf)
            b1cat[:, e * NF : (e + 1) * NF] = colmaj(e_b1[l][idx[e]])
        shared[f"w1T{l}"] = w1stk
        shared[f"b1_{l}"] = b1cat

        # w2T layout [ND, 2, 4, P, NF//4, P]:
        # [o, e, q, p, oo, c] = w2T_e_scaled[(q*8+oo)*P + p, o*P + c]
        w2stk = np.empty((ND, 2, 4, P, NF // 4, P), dtype=bf)
        for e in range(2):
            w2Ts = (w2top[e] * e_w2[l][idx[e]]).T  # [4096, 1024]
            t4 = _tile4(w2Ts, ND, NF)  # [ND, P, NF, P]
            for q4 in range(4):
                w2stk[:, e, q4] = t4[:, :, q4 * (NF // 4) : (q4 + 1) * (NF // 4), :]
        shared[f"w2T{l}"] = w2stk.astype(bf)
        b2tot = w2top[0] * e_b2[l][idx[0]] + w2top[1] * e_b2[l][idx[1]]
        shared[f"b2_{l}"] = colmaj(b2tot)
    shared["lnfg"] = colmaj(ln_f_g)
    shared["lnfb"] = colmaj(ln_f_b)

    head_pad = np.zeros((NCORES * VS, D), dtype=f32)
    head_pad[:V] = head_w

    mask_by_c = []
    for c in range(4):
        q = np.arange(TC)[None, :] + TC * c
        out = np.empty((NKV, P, TC), dtype=bf)
        for m in range(NKV):
            kv = np.arange(P)[:, None] + P * m
            out[m] = (q >= kv).astype(bf)
        mask_by_c.append(out)

    in_maps = []
    for r in range(NCORES):
        b, c = r // 4, r % 4
        m = dict(shared)
        m["toks"] = np.ascontiguousarray(
            tokens[b, TC * c : TC * (c + 1)].reshape(NT, P, 1)
        )
        m["posT"] = np.ascontiguousarray(pos_w[TC * c : TC * (c + 1)].T).astype(f32)
        m["mask"] = mask_by_c[c]
        m["headT"] = np.ascontiguousarray(head_pad[VS * r : VS * (r + 1)].T).astype(bf)
        in_maps.append(m)
    return in_maps


def kernel(**inputs):
    in_maps = _prep_inputs(inputs)
    nc = _build_program()
    res = run_bass_kernel_spmd(nc, in_maps, core_ids=list(range(NCORES)))
    B = 2
    out = np.empty((B * T, V), dtype=np.float32)
    for r in range(NCORES):
        part = res.results[r]["logits"].astype(np.float32)
        lo = VS * r
        hi = min(VS * (r + 1), V)
        out[:, lo:hi] = part[:, : hi - lo]
    return out.reshape(B, T, V)
